# revision 6
# baseline (speedup 1.0000x reference)
"""Two-layer GCN (PyG GCNConv semantics) on 8 Trainium2 NeuronCores.

v2 design. Nodes are partitioned into 8 contiguous blocks of B=12544
(padded N=100352); core c owns node block c and all edges whose dst is in
the block. gcn_conv is rewritten gather-friendly:

    hhat = d_inv[:,None] * (x @ W)
    out[v] = relu( d_inv[v] * ( sum_{e: dst=v} hhat[src_e] + hhat[v] ) + b )

Key structure (vs v1):
  - NSUB=6 source sub-tables (less chunk padding: K=3 typical vs 5).
  - s-outer processing: for each sub-table leg s, gather+aggregate all
    tiles, accumulating in an SBUF accumulator. No head-of-line blocking
    of the Pool engine on late AllGather chunks.
  - Layer-1 aggregation is TRANSPOSED (PSUM [C, slots]) so the layer-2
    local matmul gets z1^T for free (no PE transposes).
  - All matmuls in bf16 (layer-2 gathered fp32 rows are cast to bf16).
  - Self-loops are accumulator initializations, not identity matmuls.
  - Gathers grouped 8 dst-tiles per dma_gather on rotating SWDGE queues.
"""

import os
import sys

sys.path.insert(0, "/opt/trn_rl_repo")

import numpy as np

import concourse.bacc as bacc
import concourse.tile as tile
from concourse import bass_utils, mybir
from concourse.library_config import mlp

# ---------------------------------------------------------------------------
# Tile assigns Pool-engine (SWDGE) DMAs to the 8 DMASW semaphore lanes
# round-robin, ignoring queue_num. DMAs on different SWDGE queues complete
# out of order relative to each other, so a lane shared by two queues makes
# the cumulative wait thresholds unsound. Patch the lane assignment so each
# queue owns two dedicated lanes.
import concourse.tile_sem_assignment as _tsa
from concourse.tile_scheduler import DMAInst as _DMAInst

if not getattr(_tsa.TileClockTick, "_qaware_patched", False):
    _orig_assign_tick = _tsa.TileClockTick._assign_tick

    def _assign_tick_qaware(self, inst):
        from concourse import bass_isa as _bisa, mybir as _mb
        if (
            isinstance(inst, _DMAInst)
            and not isinstance(inst, _bisa.UserSyncedRemoteDMADescs)
            and inst.engine == _mb.EngineType.Pool
            and self.swdge_sem_count == 8
        ):
            q = int(getattr(inst, "queue_num", 0) or 0) % 4
            cnts = getattr(self, "_q_lane_cnt", None)
            if cnts is None:
                cnts = self._q_lane_cnt = [0, 0, 0, 0]
            self.next_sw_dma_idx = q * 2 + (cnts[q] % 2)
            cnts[q] += 1
        return _orig_assign_tick(self, inst)

    _tsa.TileClockTick._assign_tick = _assign_tick_qaware
    _tsa.TileClockTick._qaware_patched = True
# ---------------------------------------------------------------------------

# ---------------------------------------------------------------- constants
N = 100000
CIN, CHID, COUT = 128, 128, 64
NCORES = 8
B = 12544                  # nodes per core (98 tiles of 128)
NP = NCORES * B            # padded node count = 100352
TILES = B // 128           # 98 dst tiles per core
NSUB = 6                   # source sub-tables == AllGather chunks
TPC = [17, 17, 16, 16, 16, 16]   # tiles per sub-table chunk (sum = 98)
TSTART = [0, 17, 34, 50, 66, 82]
GPAIR = 8                  # dst tiles grouped per dma_gather
NGRP = -(-TILES // GPAIR)  # 13 gather groups per leg

_F32 = mybir.dt.float32
_BF16 = mybir.dt.bfloat16
_I16 = mybir.dt.int16
_CUM_TPC = np.cumsum(TPC)


def _chunk_of_tile(t):
    return int(np.searchsorted(_CUM_TPC, t, side="right"))


# ---------------------------------------------------------------- host prep
def _prep(edge_index):
    """Partition + bucket edges (s-major layout); build per-core idx/dstl
    arrays and the static leg/group/tile schedule shared by all cores."""
    src = edge_index[0].astype(np.int64)
    dst = edge_index[1].astype(np.int64)

    deg = np.bincount(dst, minlength=NP).astype(np.float32) + 1.0
    dinv = 1.0 / np.sqrt(deg)

    core = dst // B
    dstl = dst - core * B
    t = dstl >> 7                               # dst tile
    slot = dstl & 127                           # one-hot column

    csrc = src // B
    lsrc = src - csrc * B
    tsrc = lsrc >> 7
    psrc = lsrc & 127
    s = np.searchsorted(_CUM_TPC, tsrc, side="right")
    tpc_arr = np.asarray(TPC)
    tstart_arr = np.asarray(TSTART)
    row = csrc * (tpc_arr[s] * 128) + (tsrc - tstart_arr[s]) * 128 + psrc

    # s-major group order: (core, s, t)
    gid = (core * NSUB + s) * TILES + t
    order = np.argsort(gid, kind="stable")
    gid_s = gid[order]
    row_s = row[order]
    slot_s = slot[order]

    cnt = np.bincount(gid_s, minlength=NCORES * NSUB * TILES).reshape(
        NCORES, NSUB, TILES)
    K = np.maximum(1, -(-cnt.max(axis=0) // 128))            # [NSUB, TILES]
    n_chunks = int(K.sum())
    kmax = int(K.max())

    g_slot_base = np.zeros((NSUB, TILES), np.int64)
    g_chunk_base = np.zeros((NSUB, TILES), np.int64)
    acc_s = acc_c = 0
    for ss in range(NSUB):
        for tt in range(TILES):
            g_slot_base[ss, tt] = acc_s
            g_chunk_base[ss, tt] = acc_c
            acc_s += K[ss, tt] * 128
            acc_c += K[ss, tt]
    total_slots = acc_s

    grp_start = np.zeros(NCORES * NSUB * TILES + 1, np.int64)
    np.cumsum(cnt.reshape(-1), out=grp_start[1:])
    pos = np.arange(len(gid_s)) - grp_start[gid_s]

    idx_arrs, dstl_arrs = [], []
    w16 = total_slots // 16
    for c in range(NCORES):
        mask = (gid_s // (NSUB * TILES)) == c
        g_local = gid_s[mask] - c * NSUB * TILES
        ss = g_local // TILES
        tt = g_local % TILES
        flat = g_slot_base[ss, tt] + pos[mask]

        idx_flat = np.zeros(total_slots, np.int16)            # pad -> row 0
        dstl_flat = np.full(total_slots, 255.0, np.float32)   # pad -> no-op
        idx_flat[flat] = row_s[mask].astype(np.int16)
        dstl_flat[flat] = slot_s[mask].astype(np.float32)

        iw = idx_flat.reshape(w16, 16).T                      # [16, w16]
        idx_arrs.append(np.tile(iw, (8, 1)).astype(np.int16))
        dstl_arrs.append(
            np.ascontiguousarray(dstl_flat.reshape(n_chunks, 128).T))

    # schedule: per leg s, per gather group: (slot_base, kp,
    # [(t, cb, off, kk) ...])
    sched = []
    for ss in range(NSUB):
        grps = []
        for gi in range(NGRP):
            t0 = gi * GPAIR
            t1 = min(t0 + GPAIR, TILES)
            sb = int(g_slot_base[ss, t0])
            kp = int(K[ss, t0:t1].sum())
            tl = []
            for tt in range(t0, t1):
                cb = int(g_chunk_base[ss, tt])
                off = int(K[ss, t0:tt].sum())
                tl.append((tt, cb, off, int(K[ss, tt])))
            grps.append((sb, kp, tl))
        sched.append(grps)
    kp_max = max(kp for grps in sched for (_, kp, _) in grps)

    dinv_cols = [
        np.ascontiguousarray(dinv[c * B:(c + 1) * B].reshape(TILES, 128).T)
        for c in range(NCORES)
    ]
    dinv_rows = [dinv[c * B:(c + 1) * B] for c in range(NCORES)]
    return (idx_arrs, dstl_arrs, dinv_cols, dinv_rows, sched,
            n_chunks, total_slots, kmax, kp_max)


# ---------------------------------------------------------------- device IR
def _build(sched, n_chunks, total_slots, kmax, kp_max):
    nc = bacc.Bacc(
        "TRN2",
        target_bir_lowering=False,
        debug=False,
        num_devices=NCORES,
        num_swdge_queues=4,
    )

    w16 = total_slots // 16
    xt_t = nc.dram_tensor("xt", [128, B], _BF16, kind="ExternalInput")
    xt2_t = nc.dram_tensor("xt2", [128, B], _BF16, kind="ExternalInput")
    idx_t = nc.dram_tensor("idx", [128, w16], _I16, kind="ExternalInput")
    dstl_t = nc.dram_tensor("dstl", [128, n_chunks], _BF16,
                            kind="ExternalInput")
    dinv_t = nc.dram_tensor("dinv", [128, TILES], _F32, kind="ExternalInput")
    dinvrep_t = nc.dram_tensor("dinvrep", [128, B], _BF16,
                               kind="ExternalInput")
    w1_t = nc.dram_tensor("w1", [CIN, CHID], _BF16, kind="ExternalInput")
    w2_t = nc.dram_tensor("w2", [CHID, COUT], _BF16, kind="ExternalInput")
    b1c_t = nc.dram_tensor("b1c", [128, 1], _F32, kind="ExternalInput")
    b2t_t = nc.dram_tensor("b2t", [128, COUT], _F32, kind="ExternalInput")
    iotar_t = nc.dram_tensor("iotar", [128, kmax * 128], _BF16,
                             kind="ExternalInput")
    z_t = nc.dram_tensor("z", [B, COUT], _F32, kind="ExternalOutput")

    rg = [list(range(NCORES))]

    with tile.TileContext(nc) as tc:
        with (
            tc.tile_pool(name="const", bufs=1) as cpool,
            tc.tile_pool(name="xin", bufs=3) as xpool,
            tc.tile_pool(name="sel", bufs=6) as spool,
            tc.tile_pool(name="g1", bufs=3) as g1pool,
            tc.tile_pool(name="g2", bufs=3) as g2pool,
            tc.tile_pool(name="gc", bufs=3) as gcpool,
            tc.tile_pool(name="zeps", bufs=4) as zpool,
            tc.tile_pool(name="ps", bufs=2, space="PSUM") as ppool,
            tc.tile_pool(name="dram", bufs=1, space="DRAM") as dpool,
        ):
            nc.gpsimd.load_library(mlp)

            # ---- constants / inputs staged once
            idx_sb = cpool.tile([128, w16], _I16)
            nc.sync.dma_start(idx_sb[:], idx_t[:])
            dstl_sb = cpool.tile([128, n_chunks], _BF16)
            nc.sync.dma_start(dstl_sb[:], dstl_t[:])
            dinv_sb = cpool.tile([128, TILES], _F32)
            nc.sync.dma_start(dinv_sb[:], dinv_t[:])
            dinvrep_sb = cpool.tile([128, B], _BF16)
            nc.sync.dma_start(dinvrep_sb[:], dinvrep_t[:])
            w1_sb = cpool.tile([CIN, CHID], _BF16)
            nc.sync.dma_start(w1_sb[:], w1_t[:])
            w2_sb = cpool.tile([CHID, COUT], _BF16)
            nc.sync.dma_start(w2_sb[:], w2_t[:])
            b1c_sb = cpool.tile([128, 1], _F32)
            nc.sync.dma_start(b1c_sb[:], b1c_t[:])
            b2t_sb = cpool.tile([128, COUT], _F32)
            nc.sync.dma_start(b2t_sb[:], b2t_t[:])
            iota_sb = cpool.tile([128, kmax * 128], _BF16)
            nc.sync.dma_start(iota_sb[:], iotar_t[:])

            # accumulators
            acc1 = cpool.tile([128, B], _F32)            # transposed L1 agg
            acc2 = cpool.tile([128, TILES * COUT], _F32)  # L2 agg (= hh2)

            # ---- DRAM buffers: AG inputs (local hhat) and gather tables
            agin1 = [dpool.tile([TPC[s] * 128, CHID], _BF16,
                                name=f"agin1_{s}") for s in range(NSUB)]
            h1tab = [dpool.tile([NCORES * TPC[s] * 128, CHID], _BF16,
                                name=f"h1tab_{s}") for s in range(NSUB)]
            agin2 = [dpool.tile([TPC[s] * 128, COUT], _F32,
                                name=f"agin2_{s}") for s in range(NSUB)]
            h2tab = [dpool.tile([NCORES * TPC[s] * 128, COUT], _F32,
                                name=f"h2tab_{s}") for s in range(NSUB)]

            # ---------------- phase 1: hhat1 (table rows) + acc1 init
            # hh[v,:]  = dinv[v] * (x @ W1)[v]      (bf16 rows -> agin1)
            # acc1[:,v] = dinv[v] * (x @ W1)^T[:,v] (f32, = dinv * hhat^T)
            # via two matmuls on xts (plain) and xts2 (dinv-prescaled).
            for t in range(TILES):
                xts = xpool.tile([128, 128], _BF16, tag="xts")
                nc.sync.dma_start(xts[:], xt_t[:, t * 128:(t + 1) * 128])
                xts2 = xpool.tile([128, 128], _BF16, tag="xts2")
                nc.sync.dma_start(xts2[:], xt2_t[:, t * 128:(t + 1) * 128])
                psN = ppool.tile([128, CHID], _F32, tag="pagg", bufs=4)
                nc.tensor.matmul(out=psN[:], lhsT=xts[:], rhs=w1_sb[:],
                                 start=True, stop=True)
                psT = ppool.tile([128, 128], _F32, tag="pagg", bufs=4)
                nc.tensor.matmul(out=psT[:], lhsT=w1_sb[:], rhs=xts2[:],
                                 start=True, stop=True)
                hh = zpool.tile([128, CHID], _BF16, tag="hh1")
                nc.vector.tensor_scalar(
                    out=hh[:], in0=psN[:], scalar1=dinv_sb[:, t:t + 1],
                    scalar2=None, op0=mybir.AluOpType.mult)
                nc.vector.tensor_copy(acc1[:, t * 128:(t + 1) * 128], psT[:])
                s = _chunk_of_tile(t)
                r0 = (t - TSTART[s]) * 128
                nc.sync.dma_start(agin1[s][r0:r0 + 128, :], hh[:])

            for s in range(NSUB):
                nc.gpsimd.collective_compute(
                    "AllGather", mybir.AluOpType.bypass, replica_groups=rg,
                    ins=[agin1[s].opt()], outs=[h1tab[s].opt()])

            # ---------------- shared selection-matrix builder
            def build_sel(cb, kk):
                sel = spool.tile([128, kmax, 128], _BF16, tag="sel")
                nc.vector.tensor_tensor(
                    out=sel[:, :kk, :],
                    in0=iota_sb[:, :kk * 128].rearrange(
                        "p (k c) -> p k c", c=128),
                    in1=dstl_sb[:, cb:cb + kk].to_broadcast([128, kk, 128]),
                    op=mybir.AluOpType.is_equal)
                return sel

            # ---------------- layer-1 epilogue (per tile, after last leg)
            # z1T = relu(acc1 * dinvrep + b1);  hh2 = dinv * (z1 @ W2)
            def epi1(t):
                u = zpool.tile([128, 128], _F32, tag="u1")
                nc.vector.tensor_tensor(
                    out=u[:], in0=acc1[:, t * 128:(t + 1) * 128],
                    in1=dinvrep_sb[:, t * 128:(t + 1) * 128],
                    op=mybir.AluOpType.mult)
                z1T = zpool.tile([128, 128], _BF16, tag="z1T")
                nc.scalar.activation(
                    z1T[:], u[:], mybir.ActivationFunctionType.Relu,
                    bias=b1c_sb[:, 0:1])
                ps2 = ppool.tile([128, COUT], _F32, tag="pagg2", bufs=4)
                nc.tensor.matmul(out=ps2[:], lhsT=z1T[:], rhs=w2_sb[:],
                                 start=True, stop=True)
                # acc2 slice doubles as the local-hhat2 (self-loop) init
                nc.vector.tensor_scalar(
                    out=acc2[:, t * COUT:(t + 1) * COUT], in0=ps2[:],
                    scalar1=dinv_sb[:, t:t + 1],
                    scalar2=None, op0=mybir.AluOpType.mult)
                s2 = _chunk_of_tile(t)
                r0 = (t - TSTART[s2]) * 128
                nc.sync.dma_start(agin2[s2][r0:r0 + 128, :],
                                  acc2[:, t * COUT:(t + 1) * COUT])

            # ---------------- layer-2 epilogue -> output
            def epi2(t):
                v = zpool.tile([128, COUT], _F32, tag="v2")
                nc.vector.tensor_scalar(
                    out=v[:], in0=acc2[:, t * COUT:(t + 1) * COUT],
                    scalar1=dinv_sb[:, t:t + 1],
                    scalar2=None, op0=mybir.AluOpType.mult)
                nc.vector.tensor_tensor(
                    out=v[:], in0=v[:], in1=b2t_sb[:],
                    op=mybir.AluOpType.add)
                z2 = zpool.tile([128, COUT], _F32, tag="z2")
                nc.scalar.activation(
                    z2[:], v[:], mybir.ActivationFunctionType.Relu)
                nc.sync.dma_start(z_t[t * 128:(t + 1) * 128, :], z2[:])

            # ---------------- layer 1 aggregation: s-outer legs, transposed
            qctr = [0]
            for s in range(NSUB):
                for (sb, kp, tl) in sched[s]:
                    g1 = g1pool.tile([128, kp_max, CHID], _BF16, tag="g1")
                    o16 = sb // 16
                    nc.gpsimd.dma_gather(
                        g1[:, :kp, :], h1tab[s][:],
                        idx_sb[:, o16:o16 + kp * 8],
                        kp * 128, kp * 128, CHID,
                        single_packet=False, queue_num=qctr[0] % 4)
                    qctr[0] += 1
                    for (t, cb, off, kk) in tl:
                        sel = build_sel(cb, kk)
                        ps = ppool.tile([128, 128], _F32, tag="pagg", bufs=4)
                        for j in range(kk):
                            nc.tensor.matmul(
                                out=ps[:], lhsT=g1[:, off + j, :],
                                rhs=sel[:, j, :],
                                start=(j == 0), stop=(j == kk - 1))
                        nc.vector.tensor_tensor(
                            out=acc1[:, t * 128:(t + 1) * 128], in0=ps[:],
                            in1=acc1[:, t * 128:(t + 1) * 128],
                            op=mybir.AluOpType.add)
                        if s == NSUB - 1:
                            epi1(t)

            for s in range(NSUB):
                nc.gpsimd.collective_compute(
                    "AllGather", mybir.AluOpType.bypass, replica_groups=rg,
                    ins=[agin2[s].opt()], outs=[h2tab[s].opt()])

            # ---------------- layer 2 aggregation: s-outer legs
            for s in range(NSUB):
                for (sb, kp, tl) in sched[s]:
                    g2 = g2pool.tile([128, kp_max, COUT], _F32, tag="g2")
                    o16 = sb // 16
                    nc.gpsimd.dma_gather(
                        g2[:, :kp, :], h2tab[s][:],
                        idx_sb[:, o16:o16 + kp * 8],
                        kp * 128, kp * 128, COUT,
                        single_packet=False, queue_num=qctr[0] % 4)
                    qctr[0] += 1
                    gc = gcpool.tile([128, kp_max, COUT], _BF16, tag="gc")
                    nc.vector.tensor_copy(gc[:, :kp, :], g2[:, :kp, :])
                    for (t, cb, off, kk) in tl:
                        sel = build_sel(cb, kk)
                        ps = ppool.tile([128, COUT], _F32, tag="pagg2",
                                        bufs=4)
                        for j in range(kk):
                            nc.tensor.matmul(
                                out=ps[:], lhsT=sel[:, j, :],
                                rhs=gc[:, off + j, :],
                                start=(j == 0), stop=(j == kk - 1))
                        nc.vector.tensor_tensor(
                            out=acc2[:, t * COUT:(t + 1) * COUT], in0=ps[:],
                            in1=acc2[:, t * COUT:(t + 1) * COUT],
                            op=mybir.AluOpType.add)
                        if s == NSUB - 1:
                            epi2(t)

    nc.compile()
    return nc


# ---------------------------------------------------------------- entry
_last_results = None


def kernel(x, edge_index, W1, b1, W2, b2):
    global _last_results
    import ml_dtypes

    bf16 = ml_dtypes.bfloat16
    x = np.asarray(x, np.float32)
    edge_index = np.asarray(edge_index)
    W1 = np.asarray(W1, np.float32)
    b1 = np.asarray(b1, np.float32)
    W2 = np.asarray(W2, np.float32)
    b2 = np.asarray(b2, np.float32)

    (idx_arrs, dstl_arrs, dinv_cols, dinv_rows, sched,
     n_chunks, total_slots, kmax, kp_max) = _prep(edge_index)
    nc = _build(sched, n_chunks, total_slots, kmax, kp_max)

    xt = np.zeros((128, NP), np.float32)
    xt[:, :N] = x.T
    b1col = np.ascontiguousarray(b1.reshape(128, 1))
    b2_tile = np.ascontiguousarray(np.tile(b2.reshape(1, -1), (128, 1)))
    iotar_host = np.ascontiguousarray(
        np.tile(np.arange(128, dtype=np.float32), (128, kmax))).astype(bf16)
    in_maps = []
    for c in range(NCORES):
        xtc = xt[:, c * B:(c + 1) * B]
        dr = dinv_rows[c]
        in_maps.append({
            "xt": np.ascontiguousarray(xtc).astype(bf16),
            "xt2": np.ascontiguousarray(xtc * dr[None, :]).astype(bf16),
            "idx": idx_arrs[c],
            "dstl": dstl_arrs[c].astype(bf16),
            "dinv": dinv_cols[c],
            "dinvrep": np.ascontiguousarray(
                np.tile(dr.reshape(1, -1), (128, 1))).astype(bf16),
            "w1": W1.astype(bf16),
            "w2": W2.astype(bf16),
            "b1c": b1col,
            "b2t": b2_tile,
            "iotar": iotar_host,
        })

    trace = bool(os.environ.get("BASS_TRACE"))
    res = bass_utils.run_bass_kernel_spmd(
        nc, in_maps, core_ids=list(range(NCORES)), trace=trace)
    _last_results = res

    z = np.concatenate([res.results[c]["z"] for c in range(NCORES)], axis=0)
    return np.ascontiguousarray(z[:N], dtype=np.float32)


# revision 7
# speedup vs baseline: 1.1813x; 1.1813x over previous
"""Two-layer GCN (PyG GCNConv semantics) on 8 Trainium2 NeuronCores.

v3 design. Nodes are partitioned into 8 contiguous blocks of B=12544
(padded N=100352); core c owns node block c and all edges whose dst is in
the block. gcn_conv is rewritten gather-friendly:

    hhat = d_inv[:,None] * (x @ W)
    out[v] = relu( d_inv[v] * ( sum_{e: dst=v} hhat[src_e] + hhat[v] ) + b )

Structure:
  - NSUB=6 source sub-tables (K=3 chunks/tile typical), s-outer legs with
    SBUF accumulators (no head-of-line blocking on AllGather chunks).
  - Layer-1 aggregation is transposed (PSUM [C, slots]) so the layer-2
    local matmul gets z1^T for free; layer-2 is plain orientation.
  - All matmuls bf16; layer-2 gathered fp32 rows cast to bf16 on the
    Scalar (ACT) engine, keeping DVE for selection builds only.
  - One IS_EQ selection build per gather group (not per tile).
  - Self-loops are accumulator initializations (no identity matmuls).
  - Batched HWDGE DMAs: x loads 7 tiles at a time, agin writes one DMA
    per AllGather chunk, single z writeback at the end.
  - Gather tables in Shared DRAM (faster HBM-HBM AllGather).
"""

import os
import sys

sys.path.insert(0, "/opt/trn_rl_repo")

import numpy as np

import concourse.bacc as bacc
import concourse.tile as tile
from concourse import bass_utils, mybir
from concourse.library_config import mlp

# ---------------------------------------------------------------------------
# Tile assigns Pool-engine (SWDGE) DMAs to the 8 DMASW semaphore lanes
# round-robin, ignoring queue_num. DMAs on different SWDGE queues complete
# out of order relative to each other, so a lane shared by two queues makes
# the cumulative wait thresholds unsound. Patch the lane assignment so each
# queue owns two dedicated lanes.
import concourse.tile_sem_assignment as _tsa
from concourse.tile_scheduler import DMAInst as _DMAInst

if not getattr(_tsa.TileClockTick, "_qaware_patched", False):
    _orig_assign_tick = _tsa.TileClockTick._assign_tick

    def _assign_tick_qaware(self, inst):
        from concourse import bass_isa as _bisa, mybir as _mb
        if (
            isinstance(inst, _DMAInst)
            and not isinstance(inst, _bisa.UserSyncedRemoteDMADescs)
            and inst.engine == _mb.EngineType.Pool
            and self.swdge_sem_count == 8
        ):
            q = int(getattr(inst, "queue_num", 0) or 0) % 4
            cnts = getattr(self, "_q_lane_cnt", None)
            if cnts is None:
                cnts = self._q_lane_cnt = [0, 0, 0, 0]
            self.next_sw_dma_idx = q * 2 + (cnts[q] % 2)
            cnts[q] += 1
        return _orig_assign_tick(self, inst)

    _tsa.TileClockTick._assign_tick = _assign_tick_qaware
    _tsa.TileClockTick._qaware_patched = True
# ---------------------------------------------------------------------------

# ---------------------------------------------------------------- constants
N = 100000
CIN, CHID, COUT = 128, 128, 64
NCORES = 8
B = 12544                  # nodes per core (98 tiles of 128)
NP = NCORES * B            # padded node count = 100352
TILES = B // 128           # 98 dst tiles per core
NSUB = 6                   # source sub-tables == AllGather chunks
TPC = [17, 17, 16, 16, 16, 16]   # tiles per sub-table chunk (sum = 98)
TSTART = [0, 17, 34, 50, 66, 82]
GPAIR = 6                  # dst tiles grouped per dma_gather
NGRP = -(-TILES // GPAIR)  # 17 gather groups per leg
XB = 7                     # x tiles loaded per DMA (98 = 14*7)

_F32 = mybir.dt.float32
_BF16 = mybir.dt.bfloat16
_I16 = mybir.dt.int16
_CUM_TPC = np.cumsum(TPC)


def _chunk_of_tile(t):
    return int(np.searchsorted(_CUM_TPC, t, side="right"))


# ---------------------------------------------------------------- host prep
def _prep(edge_index):
    """Partition + bucket edges (s-major layout); build per-core idx/dstl
    arrays and the static leg/group/tile schedule shared by all cores."""
    src = edge_index[0].astype(np.int64)
    dst = edge_index[1].astype(np.int64)

    deg = np.bincount(dst, minlength=NP).astype(np.float32) + 1.0
    dinv = 1.0 / np.sqrt(deg)

    core = dst // B
    dstl = dst - core * B
    t = dstl >> 7                               # dst tile
    slot = dstl & 127                           # one-hot column

    csrc = src // B
    lsrc = src - csrc * B
    tsrc = lsrc >> 7
    psrc = lsrc & 127
    s = np.searchsorted(_CUM_TPC, tsrc, side="right")
    tpc_arr = np.asarray(TPC)
    tstart_arr = np.asarray(TSTART)
    row = csrc * (tpc_arr[s] * 128) + (tsrc - tstart_arr[s]) * 128 + psrc

    # s-major group order: (core, s, t)
    gid = (core * NSUB + s) * TILES + t
    order = np.argsort(gid, kind="stable")
    gid_s = gid[order]
    row_s = row[order]
    slot_s = slot[order]

    cnt = np.bincount(gid_s, minlength=NCORES * NSUB * TILES).reshape(
        NCORES, NSUB, TILES)
    K = np.maximum(1, -(-cnt.max(axis=0) // 128))            # [NSUB, TILES]
    n_chunks = int(K.sum())

    g_slot_base = np.zeros((NSUB, TILES), np.int64)
    g_chunk_base = np.zeros((NSUB, TILES), np.int64)
    acc_s = acc_c = 0
    for ss in range(NSUB):
        for tt in range(TILES):
            g_slot_base[ss, tt] = acc_s
            g_chunk_base[ss, tt] = acc_c
            acc_s += K[ss, tt] * 128
            acc_c += K[ss, tt]
    total_slots = acc_s
    leg_slot_base = [int(g_slot_base[ss, 0]) for ss in range(NSUB)]
    leg_slots = [int(K[ss].sum() * 128) for ss in range(NSUB)]

    grp_start = np.zeros(NCORES * NSUB * TILES + 1, np.int64)
    np.cumsum(cnt.reshape(-1), out=grp_start[1:])
    pos = np.arange(len(gid_s)) - grp_start[gid_s]

    idx_arrs, dstl_arrs = [], []
    w16 = total_slots // 16
    for c in range(NCORES):
        mask = (gid_s // (NSUB * TILES)) == c
        g_local = gid_s[mask] - c * NSUB * TILES
        ss = g_local // TILES
        tt = g_local % TILES
        flat = g_slot_base[ss, tt] + pos[mask]

        idx_flat = np.zeros(total_slots, np.int16)            # pad -> row 0
        dstl_flat = np.full(total_slots, 255.0, np.float32)   # pad -> no-op
        idx_flat[flat] = row_s[mask].astype(np.int16)
        dstl_flat[flat] = slot_s[mask].astype(np.float32)

        iw = idx_flat.reshape(w16, 16).T                      # [16, w16]
        idx_arrs.append(np.tile(iw, (8, 1)).astype(np.int16))
        dstl_arrs.append(
            np.ascontiguousarray(dstl_flat.reshape(n_chunks, 128).T))

    # schedule: per leg s, per gather group:
    # (slot_base, chunk_base, kp, [(t, off, kk) ...])
    sched = []
    for ss in range(NSUB):
        grps = []
        for gi in range(NGRP):
            t0 = gi * GPAIR
            t1 = min(t0 + GPAIR, TILES)
            sb = int(g_slot_base[ss, t0])
            cb = int(g_chunk_base[ss, t0])
            kp = int(K[ss, t0:t1].sum())
            tl = []
            for tt in range(t0, t1):
                off = int(K[ss, t0:tt].sum())
                tl.append((tt, off, int(K[ss, tt])))
            grps.append((sb, cb, kp, tl))
        sched.append(grps)
    kp_max = max(kp for grps in sched for (_, _, kp, _) in grps)

    dinv_cols = [
        np.ascontiguousarray(dinv[c * B:(c + 1) * B].reshape(TILES, 128).T)
        for c in range(NCORES)
    ]
    dinv_rows = [dinv[c * B:(c + 1) * B] for c in range(NCORES)]
    return (idx_arrs, dstl_arrs, dinv_cols, dinv_rows, sched,
            n_chunks, total_slots, kp_max, leg_slot_base, leg_slots)


# ---------------------------------------------------------------- device IR
def _build(sched, n_chunks, total_slots, kp_max, leg_slot_base, leg_slots):
    nc = bacc.Bacc(
        "TRN2",
        target_bir_lowering=False,
        debug=False,
        num_devices=NCORES,
        num_swdge_queues=4,
    )

    w16 = total_slots // 16
    lw16_max = max(leg_slots) // 16
    xt_t = nc.dram_tensor("xt", [128, B], _BF16, kind="ExternalInput")
    xt2_t = nc.dram_tensor("xt2", [128, B], _BF16, kind="ExternalInput")
    idx_t = nc.dram_tensor("idx", [128, w16], _I16, kind="ExternalInput")
    dstl_t = nc.dram_tensor("dstl", [128, n_chunks], _BF16,
                            kind="ExternalInput")
    dinv_t = nc.dram_tensor("dinv", [128, TILES], _F32, kind="ExternalInput")
    dinvrep_t = nc.dram_tensor("dinvrep", [128, B], _BF16,
                               kind="ExternalInput")
    w1_t = nc.dram_tensor("w1", [CIN, CHID], _BF16, kind="ExternalInput")
    w2_t = nc.dram_tensor("w2", [CHID, COUT], _BF16, kind="ExternalInput")
    b1c_t = nc.dram_tensor("b1c", [128, 1], _F32, kind="ExternalInput")
    b2t_t = nc.dram_tensor("b2t", [128, COUT], _F32, kind="ExternalInput")
    iotar_t = nc.dram_tensor("iotar", [128, kp_max * 128], _BF16,
                             kind="ExternalInput")
    z_t = nc.dram_tensor("z", [B, COUT], _F32, kind="ExternalOutput")

    rg = [list(range(NCORES))]

    with tile.TileContext(nc) as tc:
        with (
            tc.tile_pool(name="const", bufs=1) as cpool,
            tc.tile_pool(name="xin", bufs=2) as xpool,
            tc.tile_pool(name="legidx", bufs=2) as lipool,
            tc.tile_pool(name="sel", bufs=2) as spool,
            tc.tile_pool(name="g1", bufs=3) as g1pool,
            tc.tile_pool(name="g2", bufs=3) as g2pool,
            tc.tile_pool(name="gc", bufs=2) as gcpool,
            tc.tile_pool(name="hst", bufs=2) as hstpool,
            tc.tile_pool(name="zeps", bufs=4) as zpool,
            tc.tile_pool(name="ps", bufs=2, space="PSUM") as ppool,
            tc.tile_pool(name="dram", bufs=1, space="DRAM") as dpool,
        ):
            nc.gpsimd.load_library(mlp)

            # ---- constants / inputs staged once
            dstl_sb = cpool.tile([128, n_chunks], _BF16)
            nc.sync.dma_start(dstl_sb[:], dstl_t[:])
            dinv_sb = cpool.tile([128, TILES], _F32)
            nc.sync.dma_start(dinv_sb[:], dinv_t[:])
            dinvrep_sb = cpool.tile([128, B], _BF16)
            nc.sync.dma_start(dinvrep_sb[:], dinvrep_t[:])
            w1_sb = cpool.tile([CIN, CHID], _BF16)
            nc.sync.dma_start(w1_sb[:], w1_t[:])
            w2_sb = cpool.tile([CHID, COUT], _BF16)
            nc.sync.dma_start(w2_sb[:], w2_t[:])
            b1c_sb = cpool.tile([128, 1], _F32)
            nc.sync.dma_start(b1c_sb[:], b1c_t[:])
            b2t_sb = cpool.tile([128, COUT], _F32)
            nc.sync.dma_start(b2t_sb[:], b2t_t[:])
            iota_sb = cpool.tile([128, kp_max * 128], _BF16)
            nc.sync.dma_start(iota_sb[:], iotar_t[:])

            # accumulators
            acc1 = cpool.tile([128, B], _F32)            # transposed L1 agg
            acc2 = cpool.tile([128, TILES * COUT], _F32)  # L2 agg (= hh2)

            # ---- DRAM buffers: AG inputs (local hhat) and gather tables
            agin1 = [dpool.tile([TPC[s] * 128, CHID], _BF16,
                                name=f"agin1_{s}") for s in range(NSUB)]
            h1tab = [dpool.tile([NCORES * TPC[s] * 128, CHID], _BF16,
                                name=f"h1tab_{s}", addr_space="Shared")
                     for s in range(NSUB)]
            agin2 = [dpool.tile([TPC[s] * 128, COUT], _F32,
                                name=f"agin2_{s}") for s in range(NSUB)]
            h2tab = [dpool.tile([NCORES * TPC[s] * 128, COUT], _F32,
                                name=f"h2tab_{s}", addr_space="Shared")
                     for s in range(NSUB)]

            # ---------------- phase 1: hhat1 (table rows) + acc1 init
            # hh[v,:]   = dinv[v] * (x @ W1)[v]      (bf16 rows -> agin1)
            # acc1[:,v] = dinv[v] * (x @ W1)^T[:,v]  (f32)
            # x loaded 7 tiles per DMA; hh staged per AG chunk, one DMA each.
            hst = None
            for t in range(TILES):
                bi = t % XB
                if bi == 0:
                    xtb = xpool.tile([128, XB * 128], _BF16, tag="xtb")
                    nc.sync.dma_start(xtb[:],
                                      xt_t[:, t * 128:(t + XB) * 128])
                    xtb2 = xpool.tile([128, XB * 128], _BF16, tag="xtb2")
                    nc.sync.dma_start(xtb2[:],
                                      xt2_t[:, t * 128:(t + XB) * 128])
                s = _chunk_of_tile(t)
                ci = t - TSTART[s]
                if ci == 0:
                    hst = hstpool.tile([128, TPC[0] * 128], _BF16, tag="hst")
                psN = ppool.tile([128, CHID], _F32, tag="pagg", bufs=4)
                nc.tensor.matmul(out=psN[:],
                                 lhsT=xtb[:, bi * 128:(bi + 1) * 128],
                                 rhs=w1_sb[:], start=True, stop=True)
                psT = ppool.tile([128, 128], _F32, tag="pagg", bufs=4)
                nc.tensor.matmul(out=psT[:], lhsT=w1_sb[:],
                                 rhs=xtb2[:, bi * 128:(bi + 1) * 128],
                                 start=True, stop=True)
                nc.vector.tensor_scalar(
                    out=hst[:, ci * 128:(ci + 1) * 128], in0=psN[:],
                    scalar1=dinv_sb[:, t:t + 1],
                    scalar2=None, op0=mybir.AluOpType.mult)
                nc.vector.tensor_copy(acc1[:, t * 128:(t + 1) * 128], psT[:])
                if ci == TPC[s] - 1:
                    nc.sync.dma_start(
                        agin1[s][:].rearrange("(t p) c -> p t c", p=128),
                        hst[:, :TPC[s] * 128].rearrange(
                            "p (t c) -> p t c", c=CHID))
                    nc.gpsimd.collective_compute(
                        "AllGather", mybir.AluOpType.bypass,
                        replica_groups=rg,
                        ins=[agin1[s].opt()], outs=[h1tab[s].opt()])

            # ---------------- layer-1 epilogue (per tile, after last leg)
            # z1T = relu(acc1 * dinvrep + b1);  acc2 = hh2 = dinv * (z1 @ W2)
            def epi1(t):
                u = zpool.tile([128, 128], _F32, tag="u1")
                nc.vector.tensor_tensor(
                    out=u[:], in0=acc1[:, t * 128:(t + 1) * 128],
                    in1=dinvrep_sb[:, t * 128:(t + 1) * 128],
                    op=mybir.AluOpType.mult)
                z1T = zpool.tile([128, 128], _BF16, tag="z1T")
                nc.scalar.activation(
                    z1T[:], u[:], mybir.ActivationFunctionType.Relu,
                    bias=b1c_sb[:, 0:1])
                ps2 = ppool.tile([128, COUT], _F32, tag="pagg2", bufs=4)
                nc.tensor.matmul(out=ps2[:], lhsT=z1T[:], rhs=w2_sb[:],
                                 start=True, stop=True)
                nc.vector.tensor_scalar(
                    out=acc2[:, t * COUT:(t + 1) * COUT], in0=ps2[:],
                    scalar1=dinv_sb[:, t:t + 1],
                    scalar2=None, op0=mybir.AluOpType.mult)
                s2 = _chunk_of_tile(t)
                if t == TSTART[s2] + TPC[s2] - 1:
                    c0 = TSTART[s2] * COUT
                    nc.sync.dma_start(
                        agin2[s2][:].rearrange("(t p) c -> p t c", p=128),
                        acc2[:, c0:c0 + TPC[s2] * COUT].rearrange(
                            "p (t c) -> p t c", c=COUT))
                    nc.gpsimd.collective_compute(
                        "AllGather", mybir.AluOpType.bypass,
                        replica_groups=rg,
                        ins=[agin2[s2].opt()], outs=[h2tab[s2].opt()])

            # ---------------- layer-2 epilogue (writes z into acc2 in place)
            def epi2(t):
                v = zpool.tile([128, COUT], _F32, tag="v2")
                nc.vector.tensor_scalar(
                    out=v[:], in0=acc2[:, t * COUT:(t + 1) * COUT],
                    scalar1=dinv_sb[:, t:t + 1],
                    scalar2=None, op0=mybir.AluOpType.mult)
                nc.vector.tensor_tensor(
                    out=v[:], in0=v[:], in1=b2t_sb[:],
                    op=mybir.AluOpType.add)
                nc.scalar.activation(
                    acc2[:, t * COUT:(t + 1) * COUT], v[:],
                    mybir.ActivationFunctionType.Relu)

            # ---------------- aggregation legs
            qctr = [0]

            def leg(s, layer):
                li = lipool.tile([128, lw16_max], _I16, tag="lidx")
                lo16 = leg_slot_base[s] // 16
                lw = leg_slots[s] // 16
                nc.sync.dma_start(li[:, :lw], idx_t[:, lo16:lo16 + lw])
                for (sb, cb, kp, tl) in sched[s]:
                    o16 = (sb - leg_slot_base[s]) // 16
                    if layer == 1:
                        g = g1pool.tile([128, kp_max, CHID], _BF16, tag="g1")
                        nc.gpsimd.dma_gather(
                            g[:, :kp, :], h1tab[s][:],
                            li[:, o16:o16 + kp * 8],
                            kp * 128, kp * 128, CHID,
                            single_packet=False, queue_num=qctr[0] % 4)
                    else:
                        g2 = g2pool.tile([128, kp_max, COUT], _F32, tag="g2")
                        nc.gpsimd.dma_gather(
                            g2[:, :kp, :], h2tab[s][:],
                            li[:, o16:o16 + kp * 8],
                            kp * 128, kp * 128, COUT,
                            single_packet=False, queue_num=qctr[0] % 4)
                        g = gcpool.tile([128, kp_max, COUT], _BF16, tag="gc")
                        nc.scalar.copy(g[:, :kp, :], g2[:, :kp, :])
                    qctr[0] += 1
                    sel = spool.tile([128, kp_max, 128], _BF16, tag="sel")
                    nc.vector.tensor_tensor(
                        out=sel[:, :kp, :],
                        in0=iota_sb[:, :kp * 128].rearrange(
                            "p (k c) -> p k c", c=128),
                        in1=dstl_sb[:, cb:cb + kp].to_broadcast(
                            [128, kp, 128]),
                        op=mybir.AluOpType.is_equal)
                    for (t, off, kk) in tl:
                        if layer == 1:
                            ps = ppool.tile([128, 128], _F32, tag="pagg",
                                            bufs=4)
                            for j in range(kk):
                                nc.tensor.matmul(
                                    out=ps[:], lhsT=g[:, off + j, :],
                                    rhs=sel[:, off + j, :],
                                    start=(j == 0), stop=(j == kk - 1))
                            nc.vector.tensor_tensor(
                                out=acc1[:, t * 128:(t + 1) * 128],
                                in0=ps[:],
                                in1=acc1[:, t * 128:(t + 1) * 128],
                                op=mybir.AluOpType.add)
                            if s == NSUB - 1:
                                epi1(t)
                        else:
                            ps = ppool.tile([128, COUT], _F32, tag="pagg2",
                                            bufs=4)
                            for j in range(kk):
                                nc.tensor.matmul(
                                    out=ps[:], lhsT=sel[:, off + j, :],
                                    rhs=g[:, off + j, :],
                                    start=(j == 0), stop=(j == kk - 1))
                            nc.vector.tensor_tensor(
                                out=acc2[:, t * COUT:(t + 1) * COUT],
                                in0=ps[:],
                                in1=acc2[:, t * COUT:(t + 1) * COUT],
                                op=mybir.AluOpType.add)
                            if s == NSUB - 1:
                                epi2(t)

            for s in range(NSUB):
                leg(s, 1)
            for s in range(NSUB):
                leg(s, 2)

            # single batched output writeback (z lives in acc2 after epi2)
            nc.sync.dma_start(
                z_t[:].rearrange("(t p) c -> p t c", p=128),
                acc2[:].rearrange("p (t c) -> p t c", c=COUT))

    nc.compile()
    return nc


# ---------------------------------------------------------------- entry
_last_results = None


def kernel(x, edge_index, W1, b1, W2, b2):
    global _last_results
    import ml_dtypes

    bf16 = ml_dtypes.bfloat16
    x = np.asarray(x, np.float32)
    edge_index = np.asarray(edge_index)
    W1 = np.asarray(W1, np.float32)
    b1 = np.asarray(b1, np.float32)
    W2 = np.asarray(W2, np.float32)
    b2 = np.asarray(b2, np.float32)

    (idx_arrs, dstl_arrs, dinv_cols, dinv_rows, sched,
     n_chunks, total_slots, kp_max, leg_slot_base, leg_slots) = _prep(
         edge_index)
    nc = _build(sched, n_chunks, total_slots, kp_max, leg_slot_base,
                leg_slots)

    xt = np.zeros((128, NP), np.float32)
    xt[:, :N] = x.T
    b1col = np.ascontiguousarray(b1.reshape(128, 1))
    b2_tile = np.ascontiguousarray(np.tile(b2.reshape(1, -1), (128, 1)))
    iotar_host = np.ascontiguousarray(
        np.tile(np.arange(128, dtype=np.float32), (128, kp_max))).astype(bf16)
    in_maps = []
    for c in range(NCORES):
        xtc = xt[:, c * B:(c + 1) * B]
        dr = dinv_rows[c]
        in_maps.append({
            "xt": np.ascontiguousarray(xtc).astype(bf16),
            "xt2": np.ascontiguousarray(xtc * dr[None, :]).astype(bf16),
            "idx": idx_arrs[c],
            "dstl": dstl_arrs[c].astype(bf16),
            "dinv": dinv_cols[c],
            "dinvrep": np.ascontiguousarray(
                np.tile(dr.reshape(1, -1), (128, 1))).astype(bf16),
            "w1": W1.astype(bf16),
            "w2": W2.astype(bf16),
            "b1c": b1col,
            "b2t": b2_tile,
            "iotar": iotar_host,
        })

    trace = bool(os.environ.get("BASS_TRACE"))
    res = bass_utils.run_bass_kernel_spmd(
        nc, in_maps, core_ids=list(range(NCORES)), trace=trace)
    _last_results = res

    z = np.concatenate([res.results[c]["z"] for c in range(NCORES)], axis=0)
    return np.ascontiguousarray(z[:N], dtype=np.float32)


# revision 10
# speedup vs baseline: 1.2807x; 1.0842x over previous
"""Two-layer GCN (PyG GCNConv semantics) on 8 Trainium2 NeuronCores.

v3 design. Nodes are partitioned into 8 contiguous blocks of B=12544
(padded N=100352); core c owns node block c and all edges whose dst is in
the block. gcn_conv is rewritten gather-friendly:

    hhat = d_inv[:,None] * (x @ W)
    out[v] = relu( d_inv[v] * ( sum_{e: dst=v} hhat[src_e] + hhat[v] ) + b )

Structure:
  - NSUB=6 source sub-tables (K=3 chunks/tile typical), s-outer legs with
    SBUF accumulators (no head-of-line blocking on AllGather chunks).
  - Layer-1 aggregation is transposed (PSUM [C, slots]) so the layer-2
    local matmul gets z1^T for free; layer-2 is plain orientation.
  - All matmuls bf16; layer-2 gathered fp32 rows cast to bf16 on the
    Scalar (ACT) engine, keeping DVE for selection builds only.
  - One IS_EQ selection build per gather group (not per tile).
  - Self-loops are accumulator initializations (no identity matmuls).
  - Batched HWDGE DMAs: x loads 7 tiles at a time, agin writes one DMA
    per AllGather chunk, single z writeback at the end.
  - Gather tables in Shared DRAM (faster HBM-HBM AllGather).
"""

import os
import sys

sys.path.insert(0, "/opt/trn_rl_repo")

import numpy as np

import concourse.bacc as bacc
import concourse.tile as tile
from concourse import bass_utils, mybir
from concourse.library_config import mlp

# ---------------------------------------------------------------------------
# Tile assigns Pool-engine (SWDGE) DMAs to the 8 DMASW semaphore lanes
# round-robin, ignoring queue_num. DMAs on different SWDGE queues complete
# out of order relative to each other, so a lane shared by two queues makes
# the cumulative wait thresholds unsound. Patch the lane assignment so each
# queue owns two dedicated lanes.
import concourse.tile_sem_assignment as _tsa
from concourse.tile_scheduler import DMAInst as _DMAInst

if not getattr(_tsa.TileClockTick, "_qaware_patched", False):
    _orig_assign_tick = _tsa.TileClockTick._assign_tick

    def _assign_tick_qaware(self, inst):
        from concourse import bass_isa as _bisa, mybir as _mb
        if (
            isinstance(inst, _DMAInst)
            and not isinstance(inst, _bisa.UserSyncedRemoteDMADescs)
            and inst.engine == _mb.EngineType.Pool
            and self.swdge_sem_count == 8
        ):
            q = int(getattr(inst, "queue_num", 0) or 0) % 4
            cnts = getattr(self, "_q_lane_cnt", None)
            if cnts is None:
                cnts = self._q_lane_cnt = [0, 0, 0, 0]
            self.next_sw_dma_idx = q * 2 + (cnts[q] % 2)
            cnts[q] += 1
        return _orig_assign_tick(self, inst)

    _tsa.TileClockTick._assign_tick = _assign_tick_qaware
    _tsa.TileClockTick._qaware_patched = True
# ---------------------------------------------------------------------------

# ---------------------------------------------------------------- constants
N = 100000
CIN, CHID, COUT = 128, 128, 64
NCORES = 8
B = 12544                  # nodes per core (98 tiles of 128)
NP = NCORES * B            # padded node count = 100352
TILES = B // 128           # 98 dst tiles per core
NSUB = 6                   # source sub-tables == AllGather chunks
TPC = [17, 17, 16, 16, 16, 16]   # tiles per sub-table chunk (sum = 98)
TSTART = [0, 17, 34, 50, 66, 82]
GPAIR = 6                  # dst tiles grouped per dma_gather
NGRP = -(-TILES // GPAIR)  # 17 gather groups per leg
XB = 7                     # x tiles loaded per DMA (98 = 14*7)

_F32 = mybir.dt.float32
_BF16 = mybir.dt.bfloat16
_I16 = mybir.dt.int16
_CUM_TPC = np.cumsum(TPC)


def _chunk_of_tile(t):
    return int(np.searchsorted(_CUM_TPC, t, side="right"))


# ---------------------------------------------------------------- host prep
def _prep(edge_index):
    """Partition + bucket edges (s-major layout); build per-core idx/dstl
    arrays and the static leg/group/tile schedule shared by all cores."""
    src = edge_index[0].astype(np.int64)
    dst = edge_index[1].astype(np.int64)

    deg = np.bincount(dst, minlength=NP).astype(np.float32) + 1.0
    dinv = 1.0 / np.sqrt(deg)

    core = dst // B
    dstl = dst - core * B
    t = dstl >> 7                               # dst tile
    slot = dstl & 127                           # one-hot column

    csrc = src // B
    lsrc = src - csrc * B
    tsrc = lsrc >> 7
    psrc = lsrc & 127
    s = np.searchsorted(_CUM_TPC, tsrc, side="right")
    tpc_arr = np.asarray(TPC)
    tstart_arr = np.asarray(TSTART)
    row = csrc * (tpc_arr[s] * 128) + (tsrc - tstart_arr[s]) * 128 + psrc

    # s-major group order: (core, s, t)
    gid = (core * NSUB + s) * TILES + t
    order = np.argsort(gid, kind="stable")
    gid_s = gid[order]
    row_s = row[order]
    slot_s = slot[order]

    cnt = np.bincount(gid_s, minlength=NCORES * NSUB * TILES).reshape(
        NCORES, NSUB, TILES)
    K = np.maximum(1, -(-cnt.max(axis=0) // 128))            # [NSUB, TILES]
    n_chunks = int(K.sum())

    g_slot_base = np.zeros((NSUB, TILES), np.int64)
    g_chunk_base = np.zeros((NSUB, TILES), np.int64)
    acc_s = acc_c = 0
    for ss in range(NSUB):
        for tt in range(TILES):
            g_slot_base[ss, tt] = acc_s
            g_chunk_base[ss, tt] = acc_c
            acc_s += K[ss, tt] * 128
            acc_c += K[ss, tt]
    total_slots = acc_s
    leg_slot_base = [int(g_slot_base[ss, 0]) for ss in range(NSUB)]
    leg_slots = [int(K[ss].sum() * 128) for ss in range(NSUB)]

    grp_start = np.zeros(NCORES * NSUB * TILES + 1, np.int64)
    np.cumsum(cnt.reshape(-1), out=grp_start[1:])
    pos = np.arange(len(gid_s)) - grp_start[gid_s]

    idx_arrs, dstl_arrs = [], []
    w16 = total_slots // 16
    for c in range(NCORES):
        mask = (gid_s // (NSUB * TILES)) == c
        g_local = gid_s[mask] - c * NSUB * TILES
        ss = g_local // TILES
        tt = g_local % TILES
        flat = g_slot_base[ss, tt] + pos[mask]

        idx_flat = np.zeros(total_slots, np.int16)            # pad -> row 0
        dstl_flat = np.full(total_slots, 255.0, np.float32)   # pad -> no-op
        idx_flat[flat] = row_s[mask].astype(np.int16)
        dstl_flat[flat] = slot_s[mask].astype(np.float32)

        iw = idx_flat.reshape(w16, 16).T                      # [16, w16]
        idx_arrs.append(np.tile(iw, (8, 1)).astype(np.int16))
        dstl_arrs.append(
            np.ascontiguousarray(dstl_flat.reshape(n_chunks, 128).T))

    # schedule: per leg s, per gather group:
    # (slot_base, chunk_base, kp, [(t, off, kk) ...])
    sched = []
    for ss in range(NSUB):
        grps = []
        for gi in range(NGRP):
            t0 = gi * GPAIR
            t1 = min(t0 + GPAIR, TILES)
            sb = int(g_slot_base[ss, t0])
            cb = int(g_chunk_base[ss, t0])
            kp = int(K[ss, t0:t1].sum())
            tl = []
            for tt in range(t0, t1):
                off = int(K[ss, t0:tt].sum())
                tl.append((tt, off, int(K[ss, tt])))
            grps.append((sb, cb, kp, tl))
        sched.append(grps)
    kp_max = max(kp for grps in sched for (_, _, kp, _) in grps)

    dinv_cols = [
        np.ascontiguousarray(dinv[c * B:(c + 1) * B].reshape(TILES, 128).T)
        for c in range(NCORES)
    ]
    dinv_rows = [dinv[c * B:(c + 1) * B] for c in range(NCORES)]
    return (idx_arrs, dstl_arrs, dinv_cols, dinv_rows, sched,
            n_chunks, total_slots, kp_max, leg_slot_base, leg_slots)




def _cc_on_scalar(nc, kind, op, replica_groups, ins, outs):
    """Issue a collective from the Scalar engine (any engine but sync works;
    a single issuing engine preserves the straight-line collective order)."""
    type(nc.gpsimd).collective_compute(
        nc.scalar, kind, op, replica_groups=replica_groups, ins=ins,
        outs=outs)

# ---------------------------------------------------------------- device IR
def _build(sched, n_chunks, total_slots, kp_max, leg_slot_base, leg_slots):
    nc = bacc.Bacc(
        "TRN2",
        target_bir_lowering=False,
        debug=False,
        num_devices=NCORES,
        num_swdge_queues=4,
    )

    w16 = total_slots // 16
    lw16_max = max(leg_slots) // 16
    xt_t = nc.dram_tensor("xt", [128, B], _BF16, kind="ExternalInput")
    xt2_t = nc.dram_tensor("xt2", [128, B], _BF16, kind="ExternalInput")
    idx_t = nc.dram_tensor("idx", [128, w16], _I16, kind="ExternalInput")
    dstl_t = nc.dram_tensor("dstl", [128, n_chunks], _BF16,
                            kind="ExternalInput")
    dinv_t = nc.dram_tensor("dinv", [128, TILES], _F32, kind="ExternalInput")
    dinvrep_t = nc.dram_tensor("dinvrep", [128, B], _BF16,
                               kind="ExternalInput")
    w1_t = nc.dram_tensor("w1", [CIN, CHID], _BF16, kind="ExternalInput")
    w2_t = nc.dram_tensor("w2", [CHID, COUT], _BF16, kind="ExternalInput")
    b1c_t = nc.dram_tensor("b1c", [128, 1], _F32, kind="ExternalInput")
    b2t_t = nc.dram_tensor("b2t", [128, COUT], _F32, kind="ExternalInput")
    iotar_t = nc.dram_tensor("iotar", [128, kp_max * 128], _BF16,
                             kind="ExternalInput")
    z_t = nc.dram_tensor("z", [B, COUT], _F32, kind="ExternalOutput")

    rg = [list(range(NCORES))]

    with tile.TileContext(nc) as tc:
        with (
            tc.tile_pool(name="const", bufs=1) as cpool,
            tc.tile_pool(name="xin", bufs=2) as xpool,
            tc.tile_pool(name="legidx", bufs=2) as lipool,
            tc.tile_pool(name="sel", bufs=2) as spool,
            tc.tile_pool(name="g1", bufs=4) as g1pool,
            tc.tile_pool(name="g2", bufs=4) as g2pool,
            tc.tile_pool(name="gc", bufs=2) as gcpool,
            tc.tile_pool(name="hst", bufs=2) as hstpool,
            tc.tile_pool(name="zeps", bufs=4) as zpool,
            tc.tile_pool(name="ps", bufs=2, space="PSUM") as ppool,
            tc.tile_pool(name="dram", bufs=1, space="DRAM") as dpool,
        ):
            nc.gpsimd.load_library(mlp)

            # ---- constants / inputs staged once
            dstl_sb = cpool.tile([128, n_chunks], _BF16)
            nc.sync.dma_start(dstl_sb[:], dstl_t[:])
            dinv_sb = cpool.tile([128, TILES], _F32)
            nc.sync.dma_start(dinv_sb[:], dinv_t[:])
            dinvrep_sb = cpool.tile([128, B], _BF16)
            nc.sync.dma_start(dinvrep_sb[:], dinvrep_t[:])
            w1_sb = cpool.tile([CIN, CHID], _BF16)
            nc.sync.dma_start(w1_sb[:], w1_t[:])
            w2_sb = cpool.tile([CHID, COUT], _BF16)
            nc.sync.dma_start(w2_sb[:], w2_t[:])
            b1c_sb = cpool.tile([128, 1], _F32)
            nc.sync.dma_start(b1c_sb[:], b1c_t[:])
            b2t_sb = cpool.tile([128, COUT], _F32)
            nc.sync.dma_start(b2t_sb[:], b2t_t[:])
            iota_sb = cpool.tile([128, kp_max * 128], _BF16)
            nc.sync.dma_start(iota_sb[:], iotar_t[:])

            # accumulators
            acc1 = cpool.tile([128, B], _F32)            # transposed L1 agg
            acc2 = cpool.tile([128, TILES * COUT], _F32)  # L2 agg (= hh2)

            # ---- DRAM buffers: AG inputs (local hhat) and gather tables
            agin1 = [dpool.tile([TPC[s] * 128, CHID], _BF16,
                                name=f"agin1_{s}") for s in range(NSUB)]
            h1tab = [dpool.tile([NCORES * TPC[s] * 128, CHID], _BF16,
                                name=f"h1tab_{s}", addr_space="Shared")
                     for s in range(NSUB)]
            agin2 = [dpool.tile([TPC[s] * 128, COUT], _F32,
                                name=f"agin2_{s}") for s in range(NSUB)]
            h2tab = [dpool.tile([NCORES * TPC[s] * 128, COUT], _F32,
                                name=f"h2tab_{s}", addr_space="Shared")
                     for s in range(NSUB)]

            # ---------------- phase 1: hhat1 (table rows) + acc1 init
            # hh[v,:]   = dinv[v] * (x @ W1)[v]      (bf16 rows -> agin1)
            # acc1[:,v] = dinv[v] * (x @ W1)^T[:,v]  (f32)
            # x loaded 7 tiles per DMA; hh staged per AG chunk, one DMA each.
            hst = None
            for t in range(TILES):
                bi = t % XB
                if bi == 0:
                    xtb = xpool.tile([128, XB * 128], _BF16, tag="xtb")
                    nc.sync.dma_start(xtb[:],
                                      xt_t[:, t * 128:(t + XB) * 128])
                    xtb2 = xpool.tile([128, XB * 128], _BF16, tag="xtb2")
                    nc.sync.dma_start(xtb2[:],
                                      xt2_t[:, t * 128:(t + XB) * 128])
                s = _chunk_of_tile(t)
                ci = t - TSTART[s]
                if ci == 0:
                    hst = hstpool.tile([128, TPC[0] * 128], _BF16, tag="hst")
                psN = ppool.tile([128, CHID], _F32, tag="pagg", bufs=4)
                nc.tensor.matmul(out=psN[:],
                                 lhsT=xtb[:, bi * 128:(bi + 1) * 128],
                                 rhs=w1_sb[:], start=True, stop=True)
                psT = ppool.tile([128, 128], _F32, tag="pagg", bufs=4)
                nc.tensor.matmul(out=psT[:], lhsT=w1_sb[:],
                                 rhs=xtb2[:, bi * 128:(bi + 1) * 128],
                                 start=True, stop=True)
                nc.vector.tensor_scalar(
                    out=hst[:, ci * 128:(ci + 1) * 128], in0=psN[:],
                    scalar1=dinv_sb[:, t:t + 1],
                    scalar2=None, op0=mybir.AluOpType.mult)
                nc.vector.tensor_copy(acc1[:, t * 128:(t + 1) * 128], psT[:])
                if ci == TPC[s] - 1:
                    nc.sync.dma_start(
                        agin1[s][:].rearrange("(t p) c -> p t c", p=128),
                        hst[:, :TPC[s] * 128].rearrange(
                            "p (t c) -> p t c", c=CHID))
                    nc.gpsimd.collective_compute(
                        "AllGather", mybir.AluOpType.bypass,
                        replica_groups=rg,
                        ins=[agin1[s].opt()], outs=[h1tab[s].opt()])

            # ---------------- layer-1 epilogue (per tile, after last leg)
            # z1T = relu(acc1 * dinvrep + b1);  acc2 = hh2 = dinv * (z1 @ W2)
            def epi1(t):
                u = zpool.tile([128, 128], _F32, tag="u1")
                nc.vector.tensor_tensor(
                    out=u[:], in0=acc1[:, t * 128:(t + 1) * 128],
                    in1=dinvrep_sb[:, t * 128:(t + 1) * 128],
                    op=mybir.AluOpType.mult)
                z1T = zpool.tile([128, 128], _BF16, tag="z1T")
                nc.scalar.activation(
                    z1T[:], u[:], mybir.ActivationFunctionType.Relu,
                    bias=b1c_sb[:, 0:1])
                ps2 = ppool.tile([128, COUT], _F32, tag="pagg2", bufs=4)
                nc.tensor.matmul(out=ps2[:], lhsT=z1T[:], rhs=w2_sb[:],
                                 start=True, stop=True)
                nc.vector.tensor_scalar(
                    out=acc2[:, t * COUT:(t + 1) * COUT], in0=ps2[:],
                    scalar1=dinv_sb[:, t:t + 1],
                    scalar2=None, op0=mybir.AluOpType.mult)
                s2 = _chunk_of_tile(t)
                if t == TSTART[s2] + TPC[s2] - 1:
                    c0 = TSTART[s2] * COUT
                    nc.sync.dma_start(
                        agin2[s2][:].rearrange("(t p) c -> p t c", p=128),
                        acc2[:, c0:c0 + TPC[s2] * COUT].rearrange(
                            "p (t c) -> p t c", c=COUT))

            # ---------------- layer-2 epilogue (writes z into acc2 in place)
            def epi2(t):
                v = zpool.tile([128, COUT], _F32, tag="v2")
                nc.scalar.mul(v[:], acc2[:, t * COUT:(t + 1) * COUT],
                              dinv_sb[:, t:t + 1])
                nc.vector.tensor_tensor(
                    out=v[:], in0=v[:], in1=b2t_sb[:],
                    op=mybir.AluOpType.add)
                nc.scalar.activation(
                    acc2[:, t * COUT:(t + 1) * COUT], v[:],
                    mybir.ActivationFunctionType.Relu)

            # ---------------- aggregation legs
            qctr = [0]

            def leg(s, layer):
                li = lipool.tile([128, lw16_max], _I16, tag="lidx")
                lo16 = leg_slot_base[s] // 16
                lw = leg_slots[s] // 16
                nc.sync.dma_start(li[:, :lw], idx_t[:, lo16:lo16 + lw])
                for (sb, cb, kp, tl) in sched[s]:
                    o16 = (sb - leg_slot_base[s]) // 16
                    if layer == 1:
                        g = g1pool.tile([128, kp_max, CHID], _BF16, tag="g1")
                        nc.gpsimd.dma_gather(
                            g[:, :kp, :], h1tab[s][:],
                            li[:, o16:o16 + kp * 8],
                            kp * 128, kp * 128, CHID,
                            single_packet=False, queue_num=qctr[0] % 4)
                    else:
                        g2 = g2pool.tile([128, kp_max, COUT], _F32, tag="g2")
                        nc.gpsimd.dma_gather(
                            g2[:, :kp, :], h2tab[s][:],
                            li[:, o16:o16 + kp * 8],
                            kp * 128, kp * 128, COUT,
                            single_packet=False, queue_num=qctr[0] % 4)
                        g = gcpool.tile([128, kp_max, COUT], _BF16, tag="gc")
                        nc.scalar.copy(g[:, :kp, :], g2[:, :kp, :])
                    qctr[0] += 1
                    sel = spool.tile([128, kp_max, 128], _BF16, tag="sel")
                    nc.vector.tensor_tensor(
                        out=sel[:, :kp, :],
                        in0=iota_sb[:, :kp * 128].rearrange(
                            "p (k c) -> p k c", c=128),
                        in1=dstl_sb[:, cb:cb + kp].to_broadcast(
                            [128, kp, 128]),
                        op=mybir.AluOpType.is_equal)
                    for (t, off, kk) in tl:
                        if layer == 1:
                            ps = ppool.tile([128, 128], _F32, tag="pagg",
                                            bufs=4)
                            for j in range(kk):
                                nc.tensor.matmul(
                                    out=ps[:], lhsT=g[:, off + j, :],
                                    rhs=sel[:, off + j, :],
                                    start=(j == 0), stop=(j == kk - 1))
                            nc.vector.tensor_tensor(
                                out=acc1[:, t * 128:(t + 1) * 128],
                                in0=ps[:],
                                in1=acc1[:, t * 128:(t + 1) * 128],
                                op=mybir.AluOpType.add)
                            if s == NSUB - 1:
                                epi1(t)
                        else:
                            ps = ppool.tile([128, COUT], _F32, tag="pagg2",
                                            bufs=4)
                            for j in range(kk):
                                nc.tensor.matmul(
                                    out=ps[:], lhsT=sel[:, off + j, :],
                                    rhs=g[:, off + j, :],
                                    start=(j == 0), stop=(j == kk - 1))
                            nc.vector.tensor_tensor(
                                out=acc2[:, t * COUT:(t + 1) * COUT],
                                in0=ps[:],
                                in1=acc2[:, t * COUT:(t + 1) * COUT],
                                op=mybir.AluOpType.add)
                            if s == NSUB - 1:
                                epi2(t)

            for s in range(NSUB):
                leg(s, 1)
            for s in range(NSUB):
                nc.gpsimd.collective_compute(
                    "AllGather", mybir.AluOpType.bypass, replica_groups=rg,
                    ins=[agin2[s].opt()], outs=[h2tab[s].opt()])
            for s in range(NSUB):
                leg(s, 2)

            # single batched output writeback (z lives in acc2 after epi2)
            nc.sync.dma_start(
                z_t[:].rearrange("(t p) c -> p t c", p=128),
                acc2[:].rearrange("p (t c) -> p t c", c=COUT))

    nc.compile()
    return nc


# ---------------------------------------------------------------- entry
_last_results = None


def kernel(x, edge_index, W1, b1, W2, b2):
    global _last_results
    import ml_dtypes

    bf16 = ml_dtypes.bfloat16
    x = np.asarray(x, np.float32)
    edge_index = np.asarray(edge_index)
    W1 = np.asarray(W1, np.float32)
    b1 = np.asarray(b1, np.float32)
    W2 = np.asarray(W2, np.float32)
    b2 = np.asarray(b2, np.float32)

    (idx_arrs, dstl_arrs, dinv_cols, dinv_rows, sched,
     n_chunks, total_slots, kp_max, leg_slot_base, leg_slots) = _prep(
         edge_index)
    nc = _build(sched, n_chunks, total_slots, kp_max, leg_slot_base,
                leg_slots)

    xt = np.zeros((128, NP), np.float32)
    xt[:, :N] = x.T
    b1col = np.ascontiguousarray(b1.reshape(128, 1))
    b2_tile = np.ascontiguousarray(np.tile(b2.reshape(1, -1), (128, 1)))
    iotar_host = np.ascontiguousarray(
        np.tile(np.arange(128, dtype=np.float32), (128, kp_max))).astype(bf16)
    in_maps = []
    for c in range(NCORES):
        xtc = xt[:, c * B:(c + 1) * B]
        dr = dinv_rows[c]
        in_maps.append({
            "xt": np.ascontiguousarray(xtc).astype(bf16),
            "xt2": np.ascontiguousarray(xtc * dr[None, :]).astype(bf16),
            "idx": idx_arrs[c],
            "dstl": dstl_arrs[c].astype(bf16),
            "dinv": dinv_cols[c],
            "dinvrep": np.ascontiguousarray(
                np.tile(dr.reshape(1, -1), (128, 1))).astype(bf16),
            "w1": W1.astype(bf16),
            "w2": W2.astype(bf16),
            "b1c": b1col,
            "b2t": b2_tile,
            "iotar": iotar_host,
        })

    trace = bool(os.environ.get("BASS_TRACE"))
    res = bass_utils.run_bass_kernel_spmd(
        nc, in_maps, core_ids=list(range(NCORES)), trace=trace)
    _last_results = res

    z = np.concatenate([res.results[c]["z"] for c in range(NCORES)], axis=0)
    return np.ascontiguousarray(z[:N], dtype=np.float32)
